# revision 1
# baseline (speedup 1.0000x reference)
"""Trainium2 Bass kernel for nn_MemoryBank (vq_codebook softmax).

C[b, s, t] = softmax_s(-||H[b,:,t] - units[:,s]||^2)
           = softmax_s(2*cross[s,t] - m_sq[s]),  cross = units.T @ H[b]

Strategy (8 NeuronCores, data-parallel over batch B=64 -> 8 per core):
  - bf16 3-term split GEMM (h1u1 + h1u2 + h2u1) for ~fp32-accurate logits,
    with -m_sq/2 folded in as a K=2 augmentation row (bf16 hi+lo split).
  - Layout: s on partitions (output-native), t blocks of 256 on free axis.
    Cross accumulates into PSUM, two s-blocks packed per 512-col bank.
  - Softmax without any cross-partition sum machinery:
      max:  DVE max-chain over the 4 banks + GPSIMD partition_all_reduce
            -> rank-1 matmul adds -max/2 into PSUM (exact cancellation,
            bf16 rounding of the shift is harmless).
      exp1: ACT Exp(scale=2) PSUM -> fp16 SBUF tiles.
      den:  PE ones-matmul over exp1 -> [1, 512] PSUM per bank.
      out:  ACT Ln of den; -ln(den)/2 split into bf16 hi+lo -> K=2 rank-1
            matmul into PSUM; second ACT Exp(scale=2) emits the final
            normalized probabilities directly (fp32), DMA out.
"""
import numpy as np
import ml_dtypes

import concourse.bacc as bacc
import concourse.bass as bass
import concourse.bass_isa as bass_isa
import concourse.mybir as mybir
import concourse.tile as tile
from concourse.tile import add_dep_helper

F32 = mybir.dt.float32
BF16 = mybir.dt.bfloat16
FP16 = mybir.dt.float16
AF = mybir.ActivationFunctionType
ALU = mybir.AluOpType

# Problem shape (hardcoded per harness contract)
B, D, T, S = 64, 512, 2048, 1024
NCORES = 8
B_SH = B // NCORES          # batches per core
DC = D // 128               # d chunks of 128
SBK = S // 128              # s blocks of 128 (partition dim of output)
TBL = 256                   # t block (free axis) per region
NBK = 4                     # cross banks per group (2 s-blocks each)


def build_kernel(b_sh=B_SH, t=T, tbl=TBL):
    ntb = t // tbl
    nc = bacc.Bacc(None, target_bir_lowering=False, debug=False)

    h1_d = nc.dram_tensor("h1", [b_sh, DC, 128, t], BF16, kind="ExternalInput")
    h2_d = nc.dram_tensor("h2", [b_sh, DC, 128, t], BF16, kind="ExternalInput")
    u1_d = nc.dram_tensor("u1", [DC, 128, S], BF16, kind="ExternalInput")
    u2_d = nc.dram_tensor("u2", [DC, 128, S], BF16, kind="ExternalInput")
    um_d = nc.dram_tensor("um", [2, S], BF16, kind="ExternalInput")
    c_d = nc.dram_tensor("C", [b_sh, S, t], F32, kind="ExternalOutput")

    w2 = 2 * tbl                # bank width (512 cols fp32)

    with tile.TileContext(nc) as tc:
        with (
            tc.tile_pool(name="const", bufs=1) as cpool,
            tc.tile_pool(name="hbuf", bufs=2) as hpool,
            tc.tile_pool(name="work", bufs=3) as wpool,
            tc.tile_pool(name="expp", bufs=3) as epool,
            tc.tile_pool(name="outp", bufs=3) as opool,
            tc.tile_pool(name="ps", bufs=2, space="PSUM") as ps,
            tc.tile_pool(name="pstat", bufs=1, space="PSUM") as pstat,
        ):
            # --- constants loaded once ---
            u1_sb = cpool.tile([128, DC, S], BF16, tag="u1")
            u2_sb = cpool.tile([128, DC, S], BF16, tag="u2")
            nc.sync.dma_start(u1_sb[:], u1_d.rearrange("c p s -> p c s"))
            nc.sync.dma_start(u2_sb[:], u2_d.rearrange("c p s -> p c s"))
            um_sb = cpool.tile([2, S], BF16, tag="um")
            nc.sync.dma_start(um_sb[:], um_d[:])
            ones2 = cpool.tile([2, tbl], BF16, tag="ones2")
            nc.vector.memset(ones2[:], 1.0)
            ones_1x128 = cpool.tile([1, 128], BF16, tag="ones_1x128")
            nc.vector.memset(ones_1x128[:], 1.0)
            ones_2x128 = cpool.tile([2, 128], BF16, tag="ones_2x128")
            nc.vector.memset(ones_2x128[:], 1.0)
            ones_128x1 = cpool.tile([128, 1], FP16, tag="ones_128x1")
            nc.vector.memset(ones_128x1[:], 1.0)

            for b in range(b_sh):
                h1_sb = hpool.tile([128, DC, t], BF16, tag="h1")
                h2_sb = hpool.tile([128, DC, t], BF16, tag="h2")
                nc.sync.dma_start(h1_sb[:], h1_d[b].rearrange("c p t -> p c t"))
                nc.sync.dma_start(h2_sb[:], h2_d[b].rearrange("c p t -> p c t"))

                for it in range(ntb):
                    t0 = it * tbl
                    # --- cross: 4 banks, 2 s-blocks per bank ---
                    # bank k holds s-blocks 2k (cols 0:tbl) and 2k+1 (tbl:2tbl)
                    cr = [ps.tile([128, w2], F32, tag=f"cr{k}", name=f"cr{k}",
                                  bufs=(2 if k < 3 else 1)) for k in range(NBK)]

                    for k in range(NBK):
                        for half in range(2):
                            sb = 2 * k + half
                            s0 = sb * 128
                            reg = cr[k][:, half * tbl:(half + 1) * tbl]
                            # aug row: -m_sq/2 (bf16 hi+lo). start=True only
                            # on the bank's first matmul: it marks the WHOLE
                            # 2KB zero-region pending-zero; later matmuls
                            # lazily zero-then-write their own bytes.
                            nc.tensor.matmul(
                                reg, um_sb[:, s0:s0 + 128], ones2[:],
                                start=(half == 0), stop=False,
                                skip_group_check=True,
                            )
                            for c in range(DC):
                                for i, (uu, hh) in enumerate(
                                    ((u1_sb, h1_sb), (u1_sb, h2_sb),
                                     (u2_sb, h1_sb))
                                ):
                                    nc.tensor.matmul(
                                        reg,
                                        uu[:, c, s0:s0 + 128],
                                        hh[:, c, t0:t0 + tbl],
                                        start=False, stop=False,
                                        skip_group_check=True,
                                    )

                    # --- max over s: DVE chain over banks, fold halves ---
                    acc = wpool.tile([128, w2], F32, tag="acc")
                    nc.vector.tensor_copy(acc[:], cr[0][:])
                    for k in range(1, NBK):
                        nc.vector.tensor_max(acc[:], acc[:], cr[k][:])
                    tmax = wpool.tile([128, tbl], F32, tag="tmax")
                    nc.vector.tensor_max(
                        tmax[:], acc[:, 0:tbl], acc[:, tbl:2 * tbl])
                    mbc = wpool.tile([128, tbl], F32, tag="mbc")
                    nc.gpsimd.partition_all_reduce(
                        mbc[:], tmax[:], channels=128,
                        reduce_op=bass_isa.ReduceOp.max,
                    )
                    # PSUM holds l/2, so its max is M/2 already; the rank-1
                    # shift must subtract exactly mbc (scale=2 at exp time).
                    mh = wpool.tile([1, tbl], BF16, tag="mh")
                    nc.vector.tensor_scalar_mul(mh[0:1, :], mbc[0:1, :], -1.0)

                    # --- rank-1 shift (per region) + exp1 (fp16, per bank) ---
                    exps = []
                    for k in range(NBK):
                        last_r1 = None
                        for half in range(2):
                            last_r1 = nc.tensor.matmul(
                                cr[k][:, half * tbl:(half + 1) * tbl],
                                ones_1x128[:], mh[:],
                                start=False, stop=False, skip_group_check=True,
                            )
                        ex = epool.tile([128, w2], FP16, tag=f"ex{k}",
                                        name=f"ex{k}")
                        e1 = nc.scalar.activation(
                            ex[:], cr[k][:], AF.Exp, scale=2.0)
                        # Tile's accumulate-group tracking misses the RAW dep
                        # on the second rank-1 matmul; add it explicitly.
                        add_dep_helper(e1.ins, last_r1.ins, sync=True,
                                       reason="exp1 after rank1 shift")
                        exps.append(ex)

                    # --- denominator: PE ones-matmul -> [1, 512] accumulated
                    #     over banks; fold the two half-columns -> [1, 256] ---
                    den = pstat.tile([1, w2], F32, tag="den")
                    for k in range(NBK):
                        nc.tensor.matmul(
                            den[:], ones_128x1[:], exps[k][:],
                            start=(k == 0), stop=(k == NBK - 1),
                        )
                    dcp = wpool.tile([1, w2], F32, tag="dcp")
                    nc.vector.tensor_copy(dcp[:], den[:])
                    dsum = wpool.tile([1, tbl], F32, tag="dsum")
                    nc.vector.tensor_add(
                        dsum[:], dcp[0:1, 0:tbl], dcp[0:1, tbl:2 * tbl])
                    lnden = wpool.tile([1, tbl], F32, tag="lnden")
                    nc.scalar.activation(lnden[:], dsum[:], AF.Ln)
                    # split -ln(den)/2 into bf16 hi+lo rows [1, 256] each
                    ln_hi = wpool.tile([1, tbl], BF16, tag="ln_hi")
                    ln_lo = wpool.tile([1, tbl], BF16, tag="ln_lo")
                    lnlo_f = wpool.tile([1, tbl], F32, tag="lnlo_f")
                    nc.vector.tensor_scalar_mul(ln_hi[:], lnden[:], -0.5)
                    nc.vector.scalar_tensor_tensor(
                        lnlo_f[:], lnden[:], -0.5, ln_hi[:],
                        op0=ALU.mult, op1=ALU.subtract,
                    )
                    nc.vector.tensor_copy(ln_lo[:], lnlo_f[:])

                    # --- rank-1 -ln(den)/2 (per region) + exp2 -> output ---
                    for k in range(NBK):
                        last_r2 = None
                        for half in range(2):
                            reg = cr[k][:, half * tbl:(half + 1) * tbl]
                            nc.tensor.matmul(
                                reg, ones_1x128[:], ln_hi[:],
                                start=False, stop=False, skip_group_check=True,
                            )
                            last_r2 = nc.tensor.matmul(
                                reg, ones_1x128[:], ln_lo[:],
                                start=False, stop=(half == 1),
                                skip_group_check=True,
                            )
                        ot = opool.tile([128, w2], F32, tag=f"ot{k}",
                                        name=f"ot{k}")
                        e2 = nc.scalar.activation(
                            ot[:], cr[k][:], AF.Exp, scale=2.0)
                        add_dep_helper(e2.ins, last_r2.ins, sync=True,
                                       reason="exp2 after rank2 lnden")
                        for half in range(2):
                            sb = 2 * k + half
                            nc.sync.dma_start(
                                c_d[b, sb * 128:(sb + 1) * 128, t0:t0 + tbl],
                                ot[:, half * tbl:(half + 1) * tbl],
                            )

    nc.compile()
    return nc


# ---------------------------------------------------------------- host side

_RUNNER = None


def _get_runner():
    global _RUNNER
    if _RUNNER is None:
        nc = build_kernel()
        _RUNNER = _BassPjrtRunner(nc, NCORES)
    return _RUNNER


def _split_bf16(x):
    hi = x.astype(ml_dtypes.bfloat16)
    lo = (x - hi.astype(np.float32)).astype(ml_dtypes.bfloat16)
    return hi, lo


def prep_inputs(H, units):
    H = np.ascontiguousarray(np.asarray(H, dtype=np.float32))
    U = np.ascontiguousarray(np.asarray(units, dtype=np.float32))
    h1, h2 = _split_bf16(H)
    u1, u2 = _split_bf16(U)
    msq_half = -(U.astype(np.float64) ** 2).sum(0).astype(np.float32) * 0.5
    m1 = msq_half.astype(ml_dtypes.bfloat16)
    m2 = (msq_half - m1.astype(np.float32)).astype(ml_dtypes.bfloat16)
    um = np.stack([m1, m2], 0)

    u1 = u1.reshape(DC, 128, S)
    u2 = u2.reshape(DC, 128, S)
    in_maps = []
    for c in range(NCORES):
        sl = slice(c * B_SH, (c + 1) * B_SH)
        in_maps.append({
            "h1": h1[sl].reshape(B_SH, DC, 128, T),
            "h2": h2[sl].reshape(B_SH, DC, 128, T),
            "u1": u1, "u2": u2, "um": um,
        })
    return in_maps


def kernel(H, units):
    runner = _get_runner()
    in_maps = prep_inputs(H, units)
    args = runner.prep_inputs(in_maps)
    outs = runner.run(args)
    c = np.asarray(outs[0])           # (NCORES*B_SH, S, T) concat on axis 0
    return c.reshape(B, S, T)


# ------------------------------------------------- embedded PJRT runner

class _BassPjrtRunner:
    def __init__(self, nc, n_cores):
        import jax
        from jax.sharding import Mesh, PartitionSpec
        from jax.experimental.shard_map import shard_map
        from concourse import bass2jax

        bass2jax.install_neuronx_cc_hook()
        self.n_cores = n_cores
        partition_name = (
            nc.partition_id_tensor.name if nc.partition_id_tensor else None
        )
        in_names, out_names, out_avals, zero_outs = [], [], [], []
        for alloc in nc.m.functions[0].allocations:
            if not isinstance(alloc, mybir.MemoryLocationSet):
                continue
            name = alloc.memorylocations[0].name
            if alloc.kind == "ExternalInput":
                if name != partition_name:
                    in_names.append(name)
            elif alloc.kind == "ExternalOutput":
                shape = tuple(alloc.tensor_shape)
                dtype = mybir.dt.np(alloc.dtype)
                out_names.append(name)
                out_avals.append(jax.core.ShapedArray(shape, dtype))
                zero_outs.append((shape, dtype))
        self.in_names = in_names
        self.out_names = out_names
        self.out_shapes = zero_outs
        n_params = len(in_names)
        n_outs = len(out_avals)
        all_in_names = in_names + out_names
        if partition_name is not None:
            all_in_names.append(partition_name)
        self.n_params = n_params

        def _body(*args):
            operands = list(args)
            if partition_name is not None:
                operands.append(bass2jax.partition_id_tensor())
            outs = bass2jax._bass_exec_p.bind(
                *operands,
                out_avals=tuple(out_avals),
                in_names=tuple(all_in_names),
                out_names=tuple(out_names),
                lowering_input_output_aliases=(),
                sim_require_finite=False,
                sim_require_nnan=False,
                nc=nc,
            )
            return tuple(outs)

        devices = jax.devices()[:n_cores]
        assert len(devices) == n_cores
        if n_cores == 1:
            self._fn = jax.jit(_body, keep_unused=True)
        else:
            mesh = Mesh(np.asarray(devices), ("core",))
            in_specs = (PartitionSpec("core"),) * (n_params + n_outs)
            out_specs = (PartitionSpec("core"),) * n_outs
            self._fn = jax.jit(
                shard_map(_body, mesh=mesh, in_specs=in_specs,
                          out_specs=out_specs, check_rep=False),
                keep_unused=True,
            )

    def prep_inputs(self, in_maps):
        per_core = [[np.asarray(m[n]) for n in self.in_names] for m in in_maps]
        if self.n_cores == 1:
            args = per_core[0]
        else:
            args = [
                np.concatenate([per_core[c][i] for c in range(self.n_cores)], 0)
                for i in range(self.n_params)
            ]
        zouts = []
        for (s, d) in self.out_shapes:
            full = (s[0] * self.n_cores,) + tuple(s[1:]) \
                if self.n_cores > 1 else s
            zouts.append(np.zeros(full, d))
        return args + zouts

    def run(self, args):
        import jax
        outs = self._fn(*args)
        jax.block_until_ready(outs)
        return outs



# revision 7
# speedup vs baseline: 18786.4142x; 18786.4142x over previous
"""Trainium2 Bass kernel for nn_MemoryBank (vq_codebook softmax).

C[b, s, t] = softmax_s(-||H[b,:,t] - units[:,s]||^2)
           = softmax_s(2*cross[t,s] - m_sq[s]),  cross = H[b].T @ units

Strategy (8 NeuronCores, data-parallel over batch B=64 -> 8 per core):
  - t-on-partitions layout: per tile, PSUM cr[128t, 1024s] accumulates
    l/2 = cross - m_sq/2 via bf16 3-term split GEMM (h1u1 + h1u2 + h2u1)
    with -m_sq/2 folded in as a K=2 augmentation matmul.
  - Softmax over s is then a FREE-AXIS reduction:
      max:  DVE tensor_reduce per bank -> [128,1]; bias = -2*max + 13*ln2.
      exp:  single ACT pass per bank: fp16 num' = Exp(2*in + bias) with
            accum_out giving the denominator; num' is scaled by 2^13 so
            fp16 subnormals are out of the picture.
      norm+transpose: PE matmul with lhsT = num' s-slice (fp16) and
            rhs = diag(8192/den) (fp16) -> PSUM [128s, 128t], i.e. the
            transpose to output layout and the normalization in one go.
  - DVE/ACT copy PSUM -> SBUF staging, single 512KB DMA per tile out.
"""
import numpy as np
import ml_dtypes

import concourse.bacc as bacc
import concourse.bass as bass
import concourse.mybir as mybir
import concourse.tile as tile
from concourse.tile import add_dep_helper

F32 = mybir.dt.float32
BF16 = mybir.dt.bfloat16
FP16 = mybir.dt.float16
AF = mybir.ActivationFunctionType
ALU = mybir.AluOpType

# Problem shape (hardcoded per harness contract)
B, D, T, S = 64, 512, 2048, 1024
NCORES = 8
B_SH = B // NCORES          # batches per core
DC = D // 128               # d chunks of 128
TT = 128                    # t per tile (partition dim of cross)
SB = 512                    # PSUM bank width in s (fp32)
NSB = S // SB               # 2 banks per tile
SHIFT = float(13 * np.log(2.0))   # scale num by 2^13: keeps fp16 normal
SCALE_BACK = float(2.0 ** 13)


def build_kernel(b_sh=B_SH, t=T, tt=TT):
    ntile = t // tt
    nc = bacc.Bacc(None, target_bir_lowering=False, debug=False)

    h1_d = nc.dram_tensor("h1", [b_sh, DC, 128, t], BF16, kind="ExternalInput")
    h2_d = nc.dram_tensor("h2", [b_sh, DC, 128, t], BF16, kind="ExternalInput")
    u1_d = nc.dram_tensor("u1", [DC, 128, S], BF16, kind="ExternalInput")
    u2_d = nc.dram_tensor("u2", [DC, 128, S], BF16, kind="ExternalInput")
    um_d = nc.dram_tensor("um", [2, S], BF16, kind="ExternalInput")
    id_d = nc.dram_tensor("ident", [128, 128], FP16, kind="ExternalInput")
    c_d = nc.dram_tensor("C", [b_sh, S, t], F32, kind="ExternalOutput")

    with tile.TileContext(nc) as tc:
        with (
            tc.tile_pool(name="const", bufs=1) as cpool,
            tc.tile_pool(name="hbuf", bufs=2) as hpool,
            tc.tile_pool(name="work", bufs=4) as wpool,
            tc.tile_pool(name="expp", bufs=3) as epool,
            tc.tile_pool(name="diag", bufs=3) as dpool,
            tc.tile_pool(name="outp", bufs=3) as opool,
            tc.tile_pool(name="crps", bufs=2, space="PSUM") as crps,
            tc.tile_pool(name="trps", bufs=2, space="PSUM") as trps,
        ):
            # --- constants loaded once ---
            u1_sb = cpool.tile([128, DC, S], BF16, tag="u1")
            u2_sb = cpool.tile([128, DC, S], BF16, tag="u2")
            nc.sync.dma_start(u1_sb[:], u1_d.rearrange("c p s -> p c s"))
            nc.sync.dma_start(u2_sb[:], u2_d.rearrange("c p s -> p c s"))
            um_sb = cpool.tile([2, S], BF16, tag="um")
            nc.sync.dma_start(um_sb[:], um_d[:])
            id_sb = cpool.tile([128, 128], FP16, tag="ident")
            nc.sync.dma_start(id_sb[:], id_d[:])
            ones2 = cpool.tile([2, 128], BF16, tag="ones2")
            nc.vector.memset(ones2[:], 1.0)

            # state of the software-pipelined output stage
            pending = None

            def emit_output(ctx):
                """Transpose+normalize tile ctx via PE, copy to SBUF, DMA."""
                b, t0, crs, ex, diag = ctx
                ot = opool.tile([128, 4 * NSB, tt], F32, tag="ot")
                for k in range(NSB):
                    trp = trps.tile([128, SB], F32, tag=f"tr{k}",
                                    name=f"tr{k}_{b}_{t0}")
                    trs = []
                    for q in range(4):
                        sl = 4 * k + q
                        mm = nc.tensor.matmul(
                            trp[:, q * tt:(q + 1) * tt],
                            ex[:, sl * 128:(sl + 1) * 128],
                            diag[:],
                            start=(q == 0), stop=(q == 3),
                            skip_group_check=True,
                        )
                        trs.append(mm)
                    # copy PSUM -> SBUF staging (ACT for bank0, DVE for bank1)
                    # undoing the 2^13 numerator scaling (diag = 8192/den)
                    dst = ot[:, 4 * k:4 * (k + 1), :]
                    if k == 0:
                        cp = nc.scalar.mul(dst, trp[:], 1.0 / SCALE_BACK)
                    else:
                        cp = nc.vector.tensor_scalar_mul(
                            dst, trp[:], 1.0 / SCALE_BACK)
                    for mm in trs:
                        add_dep_helper(cp.ins, mm.ins, sync=True,
                                       reason="copy after transpose mm")
                nc.sync.dma_start(
                    c_d[b].rearrange("(k p) t -> p k t", p=128)[
                        :, :, t0:t0 + tt],
                    ot[:],
                )

            for b in range(b_sh):
                h1_sb = hpool.tile([128, DC, t], BF16, tag="h1")
                h2_sb = hpool.tile([128, DC, t], BF16, tag="h2")
                nc.sync.dma_start(h1_sb[:], h1_d[b].rearrange("c p t -> p c t"))
                nc.sync.dma_start(h2_sb[:], h2_d[b].rearrange("c p t -> p c t"))

                for it in range(ntile):
                    t0 = it * tt
                    # --- cross: 2 banks of [128t, 512s] ---
                    crs = []
                    for k in range(NSB):
                        s0 = k * SB
                        cr = crps.tile([128, SB], F32, tag=f"cr{k}",
                                       name=f"cr{k}_{b}_{t0}")
                        # aug: -m_sq/2 via K=2 ones matmul (zeroes the bank)
                        nc.tensor.matmul(
                            cr[:], ones2[:], um_sb[:, s0:s0 + SB],
                            start=True, stop=False,
                        )
                        last_mm = None
                        for c in range(DC):
                            for i, (hh, uu) in enumerate(
                                ((h1_sb, u1_sb), (h1_sb, u2_sb),
                                 (h2_sb, u1_sb))
                            ):
                                last_mm = nc.tensor.matmul(
                                    cr[:],
                                    hh[:, c, t0:t0 + tt],
                                    uu[:, c, s0:s0 + SB],
                                    start=False,
                                    stop=(c == DC - 1 and i == 2),
                                )
                        crs.append((cr, last_mm))

                    # --- softmax stats on the free axis ---
                    mx = []
                    for k in range(NSB):
                        m = wpool.tile([128, 1], F32, tag=f"mx{k}")
                        r = nc.vector.tensor_reduce(
                            m[:], crs[k][0][:], axis=mybir.AxisListType.X,
                            op=ALU.max,
                        )
                        add_dep_helper(r.ins, crs[k][1].ins, sync=True,
                                       reason="max after cross group")
                        mx.append(m)
                    mall = wpool.tile([128, 1], F32, tag="mall")
                    nc.vector.tensor_max(mall[:], mx[0][:], mx[1][:])
                    bias = wpool.tile([128, 1], F32, tag="bias")
                    nc.vector.tensor_scalar(
                        bias[:], mall[:], -2.0, SHIFT,
                        op0=ALU.mult, op1=ALU.add,
                    )

                    # --- exp pass: fp16 num' + fp32 den accumulation ---
                    ex = epool.tile([128, S], FP16, tag="ex")
                    dens = []
                    for k in range(NSB):
                        dn = wpool.tile([128, 1], F32, tag=f"den{k}")
                        e = nc.scalar.activation(
                            ex[:, k * SB:(k + 1) * SB], crs[k][0][:],
                            AF.Exp, bias=bias[:], scale=2.0,
                            accum_out=dn[:],
                        )
                        add_dep_helper(e.ins, crs[k][1].ins, sync=True,
                                       reason="exp after cross group")
                        dens.append(dn)
                    dsum = wpool.tile([128, 1], F32, tag="dsum")
                    nc.vector.tensor_add(dsum[:], dens[0][:], dens[1][:])
                    rec = wpool.tile([128, 1], F32, tag="rec")
                    nc.vector.reciprocal(rec[:], dsum[:])
                    recs = wpool.tile([128, 1], F32, tag="recs")
                    nc.vector.tensor_scalar_mul(recs[:], rec[:], SCALE_BACK)
                    diag = dpool.tile([128, 128], FP16, tag="diag")
                    nc.vector.tensor_scalar_mul(diag[:], id_sb[:], recs[:])

                    ctx = (b, t0, crs, ex, diag)
                    if pending is not None:
                        emit_output(pending)
                    pending = ctx

            emit_output(pending)

    nc.compile()
    return nc


# ---------------------------------------------------------------- host side

_RUNNER = None


def _get_runner():
    global _RUNNER
    if _RUNNER is None:
        nc = build_kernel()
        _RUNNER = _BassPjrtRunner(nc, NCORES)
    return _RUNNER


def _split_bf16(x):
    hi = x.astype(ml_dtypes.bfloat16)
    lo = (x - hi.astype(np.float32)).astype(ml_dtypes.bfloat16)
    return hi, lo


def prep_inputs(H, units):
    H = np.ascontiguousarray(np.asarray(H, dtype=np.float32))
    U = np.ascontiguousarray(np.asarray(units, dtype=np.float32))
    h1, h2 = _split_bf16(H)
    u1, u2 = _split_bf16(U)
    msq_half = -(U.astype(np.float64) ** 2).sum(0).astype(np.float32) * 0.5
    m1 = msq_half.astype(ml_dtypes.bfloat16)
    m2 = (msq_half - m1.astype(np.float32)).astype(ml_dtypes.bfloat16)
    um = np.stack([m1, m2], 0)
    ident = np.eye(128, dtype=np.float16)

    u1 = u1.reshape(DC, 128, S)
    u2 = u2.reshape(DC, 128, S)
    in_maps = []
    for c in range(NCORES):
        sl = slice(c * B_SH, (c + 1) * B_SH)
        in_maps.append({
            "h1": h1[sl].reshape(B_SH, DC, 128, T),
            "h2": h2[sl].reshape(B_SH, DC, 128, T),
            "u1": u1, "u2": u2, "um": um, "ident": ident,
        })
    return in_maps


def kernel(H, units):
    runner = _get_runner()
    in_maps = prep_inputs(H, units)
    args = runner.prep_inputs(in_maps)
    outs = runner.run(args)
    c = np.asarray(outs[0])           # (NCORES*B_SH, S, T) concat on axis 0
    return c.reshape(B, S, T)


# ------------------------------------------------- embedded PJRT runner

class _BassPjrtRunner:
    def __init__(self, nc, n_cores):
        import jax
        from jax.sharding import Mesh, PartitionSpec
        from jax.experimental.shard_map import shard_map
        from concourse import bass2jax

        bass2jax.install_neuronx_cc_hook()
        self.n_cores = n_cores
        partition_name = (
            nc.partition_id_tensor.name if nc.partition_id_tensor else None
        )
        in_names, out_names, out_avals, zero_outs = [], [], [], []
        for alloc in nc.m.functions[0].allocations:
            if not isinstance(alloc, mybir.MemoryLocationSet):
                continue
            name = alloc.memorylocations[0].name
            if alloc.kind == "ExternalInput":
                if name != partition_name:
                    in_names.append(name)
            elif alloc.kind == "ExternalOutput":
                shape = tuple(alloc.tensor_shape)
                dtype = mybir.dt.np(alloc.dtype)
                out_names.append(name)
                out_avals.append(jax.core.ShapedArray(shape, dtype))
                zero_outs.append((shape, dtype))
        self.in_names = in_names
        self.out_names = out_names
        self.out_shapes = zero_outs
        n_params = len(in_names)
        n_outs = len(out_avals)
        all_in_names = in_names + out_names
        if partition_name is not None:
            all_in_names.append(partition_name)
        self.n_params = n_params

        def _body(*args):
            operands = list(args)
            if partition_name is not None:
                operands.append(bass2jax.partition_id_tensor())
            outs = bass2jax._bass_exec_p.bind(
                *operands,
                out_avals=tuple(out_avals),
                in_names=tuple(all_in_names),
                out_names=tuple(out_names),
                lowering_input_output_aliases=(),
                sim_require_finite=False,
                sim_require_nnan=False,
                nc=nc,
            )
            return tuple(outs)

        devices = jax.devices()[:n_cores]
        assert len(devices) == n_cores
        if n_cores == 1:
            self._fn = jax.jit(_body, keep_unused=True)
        else:
            mesh = Mesh(np.asarray(devices), ("core",))
            in_specs = (PartitionSpec("core"),) * (n_params + n_outs)
            out_specs = (PartitionSpec("core"),) * n_outs
            self._fn = jax.jit(
                shard_map(_body, mesh=mesh, in_specs=in_specs,
                          out_specs=out_specs, check_rep=False),
                keep_unused=True,
            )

    def prep_inputs(self, in_maps):
        per_core = [[np.asarray(m[n]) for n in self.in_names] for m in in_maps]
        if self.n_cores == 1:
            args = per_core[0]
        else:
            args = [
                np.concatenate([per_core[c][i] for c in range(self.n_cores)], 0)
                for i in range(self.n_params)
            ]
        zouts = []
        for (s, d) in self.out_shapes:
            full = (s[0] * self.n_cores,) + tuple(s[1:]) \
                if self.n_cores > 1 else s
            zouts.append(np.zeros(full, d))
        return args + zouts

    def run(self, args):
        import jax
        outs = self._fn(*args)
        jax.block_until_ready(outs)
        return outs


# revision 12
# speedup vs baseline: 19251.6843x; 1.0248x over previous
"""Trainium2 Bass kernel for nn_MemoryBank (vq_codebook softmax).

C[b, s, t] = softmax_s(-||H[b,:,t] - units[:,s]||^2)
           = softmax_s(2*cross[t,s] - m_sq[s]),  cross = H[b].T @ units

Strategy (8 NeuronCores, data-parallel over batch B=64 -> 8 per core):
  - t-on-partitions layout: per tile, PSUM cr[128t, 1024s] accumulates
    cross via bf16 3-term split GEMM (h1u1 + h1u2 + h2u1), both 512-wide
    s-banks paired under each stationary h-chunk load.
  - Softmax over s is then a FREE-AXIS reduction:
      DVE tensor_tensor_reduce adds the replicated -m_sq/2 row to PSUM,
      writes l/2 to SBUF and emits the per-t max as accum_out in the
      same pass. bias = -2*max + 13*ln2.
      ACT: single Exp pass per bank: fp16 num' = Exp(2*l/2 + bias) with
      accum_out giving the denominator; the 2^13 scaling keeps fp16
      numerators out of subnormal range.
      norm+transpose: PE matmul with lhsT = num' s-slice (fp16) and
      rhs = diag(8192/den) (fp16) -> PSUM [128s, 128t]: transpose to
      output layout and normalization in one go. The PSUM->SBUF copies
      scale by 2^-13 to undo the numerator scaling.
  - single 512KB DMA per tile out of SBUF staging.
"""
import numpy as np
import ml_dtypes

import concourse.bacc as bacc
import concourse.bass as bass
import concourse.mybir as mybir
import concourse.tile as tile
from concourse.tile import add_dep_helper

F32 = mybir.dt.float32
BF16 = mybir.dt.bfloat16
FP16 = mybir.dt.float16
AF = mybir.ActivationFunctionType
ALU = mybir.AluOpType

# Problem shape (hardcoded per harness contract)
B, D, T, S = 64, 512, 2048, 1024
NCORES = 8
B_SH = B // NCORES          # batches per core
DC = D // 128               # d chunks of 128
TT = 128                    # t per tile (partition dim of cross)
SB = 512                    # PSUM bank width in s (fp32)
NSB = S // SB               # 2 banks per tile
SHIFT = float(13 * np.log(2.0))   # scale num by 2^13: keeps fp16 normal
SCALE_BACK = float(2.0 ** 13)
NEG_INF = -3.38e38


def build_kernel(b_sh=B_SH, t=T, tt=TT):
    ntile = t // tt
    nc = bacc.Bacc(None, target_bir_lowering=False, debug=False)

    h1_d = nc.dram_tensor("h1", [b_sh, DC, 128, t], BF16, kind="ExternalInput")
    h2_d = nc.dram_tensor("h2", [b_sh, DC, 128, t], BF16, kind="ExternalInput")
    u1_d = nc.dram_tensor("u1", [DC, 128, S], BF16, kind="ExternalInput")
    u2_d = nc.dram_tensor("u2", [DC, 128, S], BF16, kind="ExternalInput")
    um_d = nc.dram_tensor("um", [2, S], BF16, kind="ExternalInput")
    id_d = nc.dram_tensor("ident", [128, 128], FP16, kind="ExternalInput")
    c_d = nc.dram_tensor("C", [b_sh, S, t], F32, kind="ExternalOutput")

    with tile.TileContext(nc) as tc:
        with (
            tc.tile_pool(name="const", bufs=1) as cpool,
            tc.tile_pool(name="hbuf", bufs=2) as hpool,
            tc.tile_pool(name="work", bufs=4) as wpool,
            tc.tile_pool(name="lgt", bufs=2) as lpool,
            tc.tile_pool(name="expp", bufs=3) as epool,
            tc.tile_pool(name="diag", bufs=3) as dpool,
            tc.tile_pool(name="outp", bufs=3) as opool,
            tc.tile_pool(name="crps", bufs=2, space="PSUM") as crps,
            tc.tile_pool(name="trps", bufs=2, space="PSUM") as trps,
        ):
            # --- constants + batch-0 h, interleaved per chunk so the first
            #     cross matmuls can start as soon as chunk 0 has landed ---
            u1c, u2c = [], []

            def load_h(b):
                tiles = []
                for c in range(DC):
                    t1 = hpool.tile([128, t], BF16, tag=f"h1c{c}")
                    t2 = hpool.tile([128, t], BF16, tag=f"h2c{c}")
                    nc.sync.dma_start(t1[:], h1_d[b, c])
                    nc.sync.dma_start(t2[:], h2_d[b, c])
                    tiles.append((t1, t2))
                return tiles

            h0tiles = []
            for c in range(DC):
                uc1 = cpool.tile([128, S], BF16, tag=f"u1c{c}")
                uc2 = cpool.tile([128, S], BF16, tag=f"u2c{c}")
                nc.sync.dma_start(uc1[:], u1_d[c])
                nc.sync.dma_start(uc2[:], u2_d[c])
                u1c.append(uc1)
                u2c.append(uc2)
                t1 = hpool.tile([128, t], BF16, tag=f"h1c{c}")
                t2 = hpool.tile([128, t], BF16, tag=f"h2c{c}")
                nc.sync.dma_start(t1[:], h1_d[0, c])
                nc.sync.dma_start(t2[:], h2_d[0, c])
                h0tiles.append((t1, t2))
            id_sb = cpool.tile([128, 128], FP16, tag="ident")
            nc.sync.dma_start(id_sb[:], id_d[:])
            um_sb = cpool.tile([2, S], BF16, tag="um")
            nc.sync.dma_start(um_sb[:], um_d[:])
            ones2 = cpool.tile([2, 128], BF16, tag="ones2")
            nc.vector.memset(ones2[:], 1.0)

            # state of the software-pipelined output stage
            pending = None

            def emit_output(ctx):
                """Transpose+normalize tile ctx via PE, copy to SBUF, DMA."""
                b, t0, ex, diag = ctx
                ot = opool.tile([128, 4 * NSB, tt], F32, tag="ot")
                for k in range(NSB):
                    trp = trps.tile([128, SB], F32, tag=f"tr{k}",
                                    name=f"tr{k}_{b}_{t0}")
                    trs = []
                    for q in range(4):
                        sl = 4 * k + q
                        mm = nc.tensor.matmul(
                            trp[:, q * tt:(q + 1) * tt],
                            ex[:, sl * 128:(sl + 1) * 128],
                            diag[:],
                            start=(q == 0), stop=(q == 3),
                            skip_group_check=True,
                        )
                        trs.append(mm)
                    # copy PSUM -> SBUF staging (ACT for bank0, DVE for bank1)
                    # undoing the 2^13 numerator scaling (diag = 8192/den)
                    dst = ot[:, 4 * k:4 * (k + 1), :]
                    if k == 0:
                        cp = nc.scalar.mul(dst, trp[:], 1.0 / SCALE_BACK)
                    else:
                        cp = nc.vector.tensor_scalar_mul(
                            dst, trp[:], 1.0 / SCALE_BACK)
                    for mm in trs:
                        add_dep_helper(cp.ins, mm.ins, sync=True,
                                       reason="copy after transpose mm")
                nc.sync.dma_start(
                    c_d[b].rearrange("(k p) t -> p k t", p=128)[
                        :, :, t0:t0 + tt],
                    ot[:],
                )

            for b in range(b_sh):
                htiles = h0tiles if b == 0 else load_h(b)

                for it in range(ntile):
                    t0 = it * tt
                    # --- cross: 2 banks of [128t, 512s]; both banks paired
                    #     under each stationary so LDWEIGHTS can hide under
                    #     2 matmuls of streaming. aug adds -m_sq/2 ---
                    crs = [crps.tile([128, SB], F32, tag=f"cr{k}",
                                     name=f"cr{k}_{b}_{t0}")
                           for k in range(NSB)]
                    for k in range(NSB):
                        nc.tensor.matmul(
                            crs[k][:], ones2[:],
                            um_sb[:, k * SB:(k + 1) * SB],
                            start=True, stop=False,
                        )
                    last_mm = [None, None]
                    for c in range(DC):
                        h1c, h2c = htiles[c]
                        for i, (hh, uuc) in enumerate(
                            ((h1c, u1c), (h1c, u2c), (h2c, u1c))
                        ):
                            for k in range(NSB):
                                last_mm[k] = nc.tensor.matmul(
                                    crs[k][:],
                                    hh[:, t0:t0 + tt],
                                    uuc[c][:, k * SB:(k + 1) * SB],
                                    start=False,
                                    stop=(c == DC - 1 and i == 2),
                                )

                    # --- max over s (free axis) per bank ---
                    mx = []
                    for k in range(NSB):
                        m = wpool.tile([128, 1], F32, tag=f"mx{k}")
                        r = nc.vector.tensor_reduce(
                            m[:], crs[k][:], axis=mybir.AxisListType.X,
                            op=ALU.max,
                        )
                        add_dep_helper(r.ins, last_mm[k].ins, sync=True,
                                       reason="max after cross group")
                        mx.append(m)
                    mall = wpool.tile([128, 1], F32, tag="mall")
                    nc.vector.tensor_max(mall[:], mx[0][:], mx[1][:])
                    bias = wpool.tile([128, 1], F32, tag="bias")
                    nc.vector.tensor_scalar(
                        bias[:], mall[:], -2.0, SHIFT,
                        op0=ALU.mult, op1=ALU.add,
                    )

                    # --- exp pass: fp16 num' + fp32 den accumulation ---
                    ex = epool.tile([128, S], FP16, tag="ex")
                    dens = []
                    for k in range(NSB):
                        dn = wpool.tile([128, 1], F32, tag=f"den{k}")
                        e = nc.scalar.activation(
                            ex[:, k * SB:(k + 1) * SB], crs[k][:],
                            AF.Exp, bias=bias[:], scale=2.0,
                            accum_out=dn[:],
                        )
                        add_dep_helper(e.ins, last_mm[k].ins, sync=True,
                                       reason="exp after cross group")
                        dens.append(dn)
                    dsum = wpool.tile([128, 1], F32, tag="dsum")
                    nc.vector.tensor_add(dsum[:], dens[0][:], dens[1][:])
                    rec = wpool.tile([128, 1], F32, tag="rec")
                    nc.vector.reciprocal(rec[:], dsum[:])
                    recs = wpool.tile([128, 1], F32, tag="recs")
                    nc.vector.tensor_scalar_mul(recs[:], rec[:], SCALE_BACK)
                    diag = dpool.tile([128, 128], FP16, tag="diag")
                    nc.vector.tensor_scalar_mul(diag[:], id_sb[:], recs[:])

                    ctx = (b, t0, ex, diag)
                    if pending is not None:
                        emit_output(pending)
                    pending = ctx

            emit_output(pending)

    nc.compile()
    return nc


# ---------------------------------------------------------------- host side

_RUNNER = None


def _get_runner():
    global _RUNNER
    if _RUNNER is None:
        nc = build_kernel()
        _RUNNER = _BassPjrtRunner(nc, NCORES)
    return _RUNNER


def _split_bf16(x):
    hi = x.astype(ml_dtypes.bfloat16)
    lo = (x - hi.astype(np.float32)).astype(ml_dtypes.bfloat16)
    return hi, lo


def prep_inputs(H, units):
    H = np.ascontiguousarray(np.asarray(H, dtype=np.float32))
    U = np.ascontiguousarray(np.asarray(units, dtype=np.float32))
    h1, h2 = _split_bf16(H)
    u1, u2 = _split_bf16(U)
    msq_half = -(U.astype(np.float64) ** 2).sum(0).astype(np.float32) * 0.5
    m1 = msq_half.astype(ml_dtypes.bfloat16)
    m2 = (msq_half - m1.astype(np.float32)).astype(ml_dtypes.bfloat16)
    um = np.stack([m1, m2], 0)
    ident = np.eye(128, dtype=np.float16)

    u1 = u1.reshape(DC, 128, S)
    u2 = u2.reshape(DC, 128, S)
    in_maps = []
    for c in range(NCORES):
        sl = slice(c * B_SH, (c + 1) * B_SH)
        in_maps.append({
            "h1": h1[sl].reshape(B_SH, DC, 128, T),
            "h2": h2[sl].reshape(B_SH, DC, 128, T),
            "u1": u1, "u2": u2, "um": um, "ident": ident,
        })
    return in_maps


def kernel(H, units):
    runner = _get_runner()
    in_maps = prep_inputs(H, units)
    args = runner.prep_inputs(in_maps)
    outs = runner.run(args)
    c = np.asarray(outs[0])           # (NCORES*B_SH, S, T) concat on axis 0
    return c.reshape(B, S, T)


# ------------------------------------------------- embedded PJRT runner

class _BassPjrtRunner:
    def __init__(self, nc, n_cores):
        import jax
        from jax.sharding import Mesh, PartitionSpec
        from jax.experimental.shard_map import shard_map
        from concourse import bass2jax

        bass2jax.install_neuronx_cc_hook()
        self.n_cores = n_cores
        partition_name = (
            nc.partition_id_tensor.name if nc.partition_id_tensor else None
        )
        in_names, out_names, out_avals, zero_outs = [], [], [], []
        for alloc in nc.m.functions[0].allocations:
            if not isinstance(alloc, mybir.MemoryLocationSet):
                continue
            name = alloc.memorylocations[0].name
            if alloc.kind == "ExternalInput":
                if name != partition_name:
                    in_names.append(name)
            elif alloc.kind == "ExternalOutput":
                shape = tuple(alloc.tensor_shape)
                dtype = mybir.dt.np(alloc.dtype)
                out_names.append(name)
                out_avals.append(jax.core.ShapedArray(shape, dtype))
                zero_outs.append((shape, dtype))
        self.in_names = in_names
        self.out_names = out_names
        self.out_shapes = zero_outs
        n_params = len(in_names)
        n_outs = len(out_avals)
        all_in_names = in_names + out_names
        if partition_name is not None:
            all_in_names.append(partition_name)
        self.n_params = n_params

        def _body(*args):
            operands = list(args)
            if partition_name is not None:
                operands.append(bass2jax.partition_id_tensor())
            outs = bass2jax._bass_exec_p.bind(
                *operands,
                out_avals=tuple(out_avals),
                in_names=tuple(all_in_names),
                out_names=tuple(out_names),
                lowering_input_output_aliases=(),
                sim_require_finite=False,
                sim_require_nnan=False,
                nc=nc,
            )
            return tuple(outs)

        devices = jax.devices()[:n_cores]
        assert len(devices) == n_cores
        if n_cores == 1:
            self._fn = jax.jit(_body, keep_unused=True)
        else:
            mesh = Mesh(np.asarray(devices), ("core",))
            in_specs = (PartitionSpec("core"),) * (n_params + n_outs)
            out_specs = (PartitionSpec("core"),) * n_outs
            self._fn = jax.jit(
                shard_map(_body, mesh=mesh, in_specs=in_specs,
                          out_specs=out_specs, check_rep=False),
                keep_unused=True,
            )

    def prep_inputs(self, in_maps):
        per_core = [[np.asarray(m[n]) for n in self.in_names] for m in in_maps]
        if self.n_cores == 1:
            args = per_core[0]
        else:
            args = [
                np.concatenate([per_core[c][i] for c in range(self.n_cores)], 0)
                for i in range(self.n_params)
            ]
        zouts = []
        for (s, d) in self.out_shapes:
            full = (s[0] * self.n_cores,) + tuple(s[1:]) \
                if self.n_cores > 1 else s
            zouts.append(np.zeros(full, d))
        return args + zouts

    def run(self, args):
        import jax
        outs = self._fn(*args)
        jax.block_until_ready(outs)
        return outs
